# revision 30
# baseline (speedup 1.0000x reference)
"""AGCN (MLP + K-step gated Laplacian propagation) on 8 TRN2 NeuronCores.

Design:
  - Nodes sharded over 8 cores; NT=100 tiles of 128 slots per core
    (12800 slots incl. ~300 fake pad slots, 12500 real nodes/core).
  - Windows = slot-quarters (tile index mod 4, 25 tiles each). Per step,
    each core scales its inp shard by dinv and runs FOUR per-quarter
    AllGathers into Shared-scratch window tables [25600, 64] f32 (fast
    collective path needs addr_space="Shared" + single writer; separate
    tensors give the tile dep-tracker per-window granularity).
  - Gather schedule: chunk = quarter q, processed q=0..3; windows ascend
    within a chunk. After quarter q's last gather call, that quarter's
    update + gate/hidden + in_cc write + AllGather(q) are emitted, so all
    vector work and collectives hide behind the next chunk's gathers and
    next step's w0-w2 gathers hide AG(q3). Everything except the gather
    stream is off the critical path (gather-only == full kernel time).
  - Slot packing: per (core, quarter) greedy vector packing (node -> tile
    minimizing sum-of-window-maxima increase), tiles relabeled desc so
    per-quarter prefix widths are tight: ~498k gathered rows/core/step vs
    400k real edges (vs 676k for the legacy lexsort+prefix).
  - Accumulation: gathered tiles summed on TensorE via identity-matmul
    into PSUM ([P,8,64] = one-bank slices keyed (quarter, slice),
    start/stop from the static coverage map) and drained to SBUF by the
    scalar engine with stride-4 APs. Keeps VectorE out of the gather
    stream (SBUF-port contention stalls GpSimd SWDGE).
  - MLP (x@W1 relu @W2) in bf16 on TensorE; x arrives bf16 from the host
    and feeds dma_start_transpose directly.
  - Gather throughput law (measured): scales with total gather-buffer
    bytes. TCH=25 tiles/call, GBUFS=14 (87.5KB/partition), IBUFS=12 is
    the sweet spot; fewer+bigger calls (TCH>=50, low GBUFS) is 1.2-2.6x
    SLOWER, and so are more+smaller calls (TCH 8-16 at equal bytes).
    4 SWDGE queues round-robin (ci%4).
  - Known device-wedging traps (do not retry): dma_gather
    single_packet=True; gpsimd dma_start(accum_op=add) SBUF->SBUF;
    KM_IDXB=1 broadcast-DMA idx loads (kills the PJRT worker).
  - benchmark()/test.py time dispatch + device completion (fetch=False);
    the axon tunnel adds ~85-100ms RTT + ~0.7ms/MB/core input streaming
    per call, so test.py interleaves kernel and same-signature floor-probe
    timing blocks and subtracts the floor.
"""

import sys

sys.path.insert(0, "/opt/trn_rl_repo")

import numpy as np
import os as _os

DIAG = _os.environ.get("KM_DIAG", "")
TCH_ENV = int(_os.environ.get("KM_TCH", "25"))
GBUFS = int(_os.environ.get("KM_GBUFS", "14"))
ACC_MODE = _os.environ.get("KM_ACC", "pe")  # dve | cce
SINGLE_PACKET = bool(int(_os.environ.get("KM_SP", "0")))
TMPB = int(_os.environ.get("KM_TMPB", "1"))
IBUFS = int(_os.environ.get("KM_IBUFS", "12"))
CH_ENV = int(_os.environ.get("KM_CH", "32"))
ORDER = _os.environ.get("KM_ORDER", "wmaj")  # wmaj | winrr
CHB = int(_os.environ.get("KM_CHB", "2"))
IDXB = bool(int(_os.environ.get("KM_IDXB", "0")))  # store idx 16-wide, bcast-DMA

N = 100000
E = 3200000
NFEAT = 512
NHID = 256
NCLASS = 64
K = 10
NCORES = 8
P = 128
NT = 100                # tiles per core
S = P * NT              # 12800 slots per core
NREAL = N // NCORES     # 12500 real nodes per core
NWIN = 4
# windows = slot-quarters (tile index mod 4) so each window's table comes
# from a separate per-quarter AllGather that overlaps next-step gathers.
# NT=100 makes all quarters equal (25 tiles / 3125 real nodes), which keeps
# the window balancer's caps uniform.
NQT = [25, 25, 25, 25]          # tiles per quarter (t % 4)
QS = [t * P for t in NQT]       # slots per quarter per core
QR = [NCORES * s for s in QS]   # window table rows (25600, int16-safe)
QCAP = [3125, 3125, 3125, 3125] # real nodes per (core, quarter); sum 12500
TCH = TCH_ENV           # tiles per gather call

_CACHE = {}


def assign_windows(src, dst, order0, N, rounds=96, seed=0):
    """Free greedy window assignment balancing every dst's in-neighbor
    window counts; per-window caps = 8*QCAP[w]; cores dealt 8-way by
    in-degree within each window. Returns (core, quarter)."""
    NWINL = NWIN
    caps = np.array([NCORES * QCAP[w] for w in range(NWINL)])
    rng = np.random.default_rng(seed)
    deg_out = np.bincount(src, minlength=N).astype(np.float32)
    win = (np.arange(N) % NWINL).astype(np.int8)
    rng.shuffle(win)
    for r in range(rounds):
        tally = np.zeros((N, NWINL), np.int32)
        for w in range(NWINL):
            tally[:, w] = np.bincount(dst[win[src] == w], minlength=N)
        score = np.zeros((N, NWINL), np.float32)
        for w in range(NWINL):
            score[:, w] = np.bincount(src, weights=tally[dst, w], minlength=N)
        score[np.arange(N), win] -= deg_out
        frac = 0.3 * (0.5 + 0.5 * (rounds - r) / rounds)
        sel = rng.random(N) < frac
        want = score.argmin(axis=1).astype(np.int8)
        move = sel & (want != win)
        neww = win.copy()
        neww[move] = want[move]
        for _ in range(4):  # iterate trim to capacity
            counts = np.bincount(neww, minlength=NWINL)
            if (counts <= caps).all():
                break
            for w in range(NWINL):
                over = counts[w] - caps[w]
                if over > 0:
                    cand = np.where(move & (neww == w) & (win != w))[0]
                    if len(cand) == 0:
                        continue
                    rv = rng.choice(cand, size=min(over, len(cand)), replace=False)
                    neww[rv] = win[rv]
        win = neww
    # exact capacity repair: move nodes from over-cap windows to max-slack
    counts = np.bincount(win, minlength=NWINL)
    for w in range(NWINL):
        while counts[w] > caps[w]:
            cand = np.where(win == w)[0]
            k = counts[w] - caps[w]
            tgt = int(np.argmax(caps - counts))
            win[cand[:k]] = tgt
            counts = np.bincount(win, minlength=NWINL)
    # cores: within window, sort by -indeg, deal round-robin over 8 cores
    indeg = np.bincount(dst, minlength=N)
    core = np.empty(N, np.int64)
    for w in range(NWINL):
        nodes_w = np.where(win == w)[0]
        nodes_w = nodes_w[np.argsort(-indeg[nodes_w], kind="stable")]
        for c in range(NCORES):
            core[nodes_w[c::NCORES]] = c
    return core, win.astype(np.int64)


# --------------------------------------------------------------------------
# host preprocessing
# --------------------------------------------------------------------------
def _preprocess(edge_index):
    src = edge_index[0].astype(np.int64)
    dst = edge_index[1].astype(np.int64)

    deg_out = np.bincount(src, minlength=N)
    dinv = np.where(deg_out > 0, 1.0 / np.sqrt(np.maximum(deg_out, 1)), 0.0).astype(
        np.float32
    )
    indeg = np.bincount(dst, minlength=N)

    # window-balanced assignment: windows = slot-quarters, chosen to
    # equalize every dst's in-neighbor window counts
    order0 = np.argsort(-indeg, kind="stable")
    core, win_node = assign_windows(src, dst, order0, N)

    # per-dst per-window in-degree
    ws_all = win_node[src]
    degw = np.stack(
        [np.bincount(dst[ws_all == w], minlength=N) for w in range(NWIN)], axis=1
    )

    # final slot assignment: per (core, quarter) bucket. "greedy" packs
    # nodes (desc by max window degree) into the tile whose sum-of-window-
    # maxima increases least, then relabels tiles desc by tile-max sum so
    # the global prefix schedule is tight. Quarter q owns tiles t%4==q.
    def _order_greedy(dw, ntile):
        n = len(dw)
        keyorder = np.argsort(-dw.max(axis=1), kind="stable")
        tmax = np.zeros((ntile, NWIN), np.int64)
        tcnt = np.zeros(ntile, np.int64)
        pos = np.empty(n, np.int64)
        for i in keyorder:
            d = dw[i]
            inc = np.maximum(d[None, :], tmax).sum(axis=1) - tmax.sum(axis=1)
            inc[tcnt >= P] = 1 << 40
            t = int(np.argmin(inc * 256 - tcnt))
            pos[i] = t * P + tcnt[t]
            tmax[t] = np.maximum(tmax[t], d)
            tcnt[t] += 1
        # relabel tiles desc by tile-max sum
        perm_t = np.argsort(-tmax.sum(axis=1), kind="stable")
        inv_t = np.empty(ntile, np.int64)
        inv_t[perm_t] = np.arange(ntile)
        return inv_t[pos // P] * P + pos % P

    slot = np.empty(N, np.int64)
    rel = np.empty(N, np.int64)
    for c in range(NCORES):
        for q in range(NWIN):
            nodes_b = np.where((core == c) & (win_node == q))[0]
            dw = degw[nodes_b].astype(np.int64)
            pos = _order_greedy(dw, NQT[q])  # within-quarter position
            kq, pq = pos // P, pos % P
            slot[nodes_b] = (kq * NWIN + q) * P + pq
            rel[nodes_b] = c * QS[q] + pos   # row within window-q table
    rel = rel.astype(np.int16)

    # ELL grids: grids[w][c, j, s] = rel row of j-th window-w in-neighbor of
    # the node at (core c, slot s); PAD_REL otherwise.
    Jmax = degw.max(axis=0)
    ek = np.lexsort((src, ws_all, dst))
    ds, ss, wse = dst[ek], src[ek], ws_all[ek]
    grp = ds * NWIN + wse
    newg = np.r_[True, grp[1:] != grp[:-1]]
    gstart = np.maximum.accumulate(np.where(newg, np.arange(E), 0))
    jr = np.arange(E) - gstart

    # actual empty (guaranteed-zero) slots per core: greedy packing can leave
    # them anywhere within each quarter's tile set
    filled = np.zeros((NCORES, S), bool)
    for c in range(NCORES):
        nodes_c = np.where(core == c)[0]
        filled[c, slot[nodes_c]] = True

    grids = []
    prng = np.random.default_rng(7)
    # pad targets: spread across all fake (guaranteed-zero) rows of the window
    # to avoid HBM same-address serialization
    fake_rels_w = []
    for w in range(NWIN):
        frs = []
        for c in range(NCORES):
            es = np.where(~filled[c])[0]
            es = es[(es // P) % NWIN == w]
            o = (es // (NWIN * P)) * P + es % P  # within-quarter row offset
            frs.append(c * QS[w] + o)
        fr = np.concatenate(frs).astype(np.int16)
        assert len(fr) > 0
        fake_rels_w.append(fr)
    # "lane" pads: gather position i is serviced by SDMA engine i%16, so a
    # fixed fake row per lane turns pad reads into repeated same-row (row-
    # buffer-hit) reads on a constant HBM channel. "rand" is the legacy
    # spread-across-all-fake-rows scheme.
    pad_lane = _os.environ.get("KM_PAD", "rand") == "lane"
    for w in range(NWIN):
        fake_rels = fake_rels_w[w]
        if pad_lane:
            lanes = fake_rels[np.arange(S) % len(fake_rels[:16])]
            g = (
                np.broadcast_to(lanes, (NCORES, Jmax[w], S))
                .copy()
                .astype(np.int16)
            )
        else:
            g = prng.choice(fake_rels, size=(NCORES, Jmax[w], S)).astype(np.int16)
        m = wse == w
        g[core[ds[m]], jr[m], slot[ds[m]]] = rel[ss[m]]
        grids.append(g)

    # per-tile max degw -> global (over cores) per-tile maxima tmg[w][t]
    tmg_all = np.zeros((NWIN, NT), np.int64)
    for w in range(NWIN):
        tm = np.zeros((NCORES, NT), np.int64)
        for c in range(NCORES):
            sd = np.zeros(S, np.int64)
            nodes_c = np.where(core == c)[0]
            sd[slot[nodes_c]] = degw[nodes_c, w]
            tm[c] = sd.reshape(NT, P).max(axis=1)
        tmg_all[w] = tm.max(axis=0)  # same for all cores (SPMD)

    # call schedule: chunk = quarter q (tiles q::4, within-quarter maxima
    # descending by relabel). A call packs MULTIPLE j-levels of one (w, q)
    # into a single dma_gather of <= TCH tiles (per-call overhead dominates
    # at small calls: 188 single-level calls ran ~19% slower per row than
    # the old 179). Call = (w, q, levels, col_off) with levels a tuple of
    # (j, kw): the j-th window-w in-neighbor of the first kw tiles of
    # quarter q. Chunks processed q=0..3, w ascending, so quarter q's
    # update + AllGather(q) fire while later chunks still gather.
    nqt = NQT[0]
    # split levels at 8-aligned tile boundaries when they exceed TCH (PSUM
    # slice keys stay aligned because parts never straddle an 8-tile slice)
    TCH8 = max(8, (TCH // 8) * 8) if TCH < nqt else nqt
    calls = []
    col_off = 0
    for q in range(NWIN):
        percall_w = []  # per window: list of level-bins
        for w in range(NWIN):
            parts = []
            for j in range(Jmax[w]):
                kw = int((tmg_all[w][q::NWIN] > j).sum())
                if w == 0 and j == 0:
                    kw = nqt  # force full-width init pass (PSUM start cover)
                if kw == 0:
                    continue
                for klo in range(0, kw, TCH8):
                    parts.append((j, klo, min(klo + TCH8, kw)))
            bins = []
            bin_levels = []
            bin_tiles = 0
            for (j, klo, khi) in parts:
                if bin_tiles + (khi - klo) > TCH:
                    bins.append(tuple(bin_levels))
                    bin_levels, bin_tiles = [], 0
                bin_levels.append((j, klo, khi))
                bin_tiles += khi - klo
            if bin_levels:
                bins.append(tuple(bin_levels))
            percall_w.append(bins)
        if ORDER == "winrr":
            # round-robin windows so concurrent queues hit different window
            # tables (spreads HBM channels); bin 0 of each window leads, so
            # next-step AG(q3) still hides behind w0-w2 bin-0 gathers.
            order_wb = [
                (w, b)
                for b in range(max(len(x) for x in percall_w))
                for w in range(NWIN)
                if b < len(percall_w[w])
            ]
        else:
            order_wb = [
                (w, b) for w in range(NWIN) for b in range(len(percall_w[w]))
            ]
        for (w, b) in order_wb:
            levels = percall_w[w][b]
            calls.append((w, q, levels, col_off))
            col_off += sum(khi - klo for (j, klo, khi) in levels) * P // 16
    C2 = col_off

    idx_rows = 16 if IDXB else P
    idxbuf = np.empty((NCORES, idx_rows, C2), np.int16)
    for c in range(NCORES):
        parts = []
        for (w, q, levels, off) in calls:
            for (j, klo, khi) in levels:
                vals = grids[w][c, j].reshape(NT, P)[q::NWIN][klo:khi].reshape(-1)
                a = vals.reshape(-1, 16).T  # [16, n/16]
                parts.append(a if IDXB else np.tile(a, (8, 1)))
        idxbuf[c] = np.concatenate(parts, axis=1)

    # per-core slot-ordered dinv [128, 98] and node<->slot maps
    dinv_slot = np.zeros((NCORES, P, NT), np.float32)
    nodemap = np.full((NCORES, S), -1, np.int64)  # slot -> node
    for c in range(NCORES):
        nodes_c = np.where(core == c)[0]
        sl = slot[nodes_c]
        nodemap[c, sl] = nodes_c
        pp, tt = sl % P, sl // P
        dinv_slot[c, pp, tt] = dinv[nodes_c]

    return {
        "calls": calls,
        "C2": C2,
        "idxbuf": idxbuf,
        "dinv_slot": dinv_slot,
        "nodemap": nodemap,
        "dinv": dinv,
    }


# --------------------------------------------------------------------------
# device kernel builder
# --------------------------------------------------------------------------
def _build_nc(calls, C2):
    from concourse import bacc, bass, mybir, tile
    from concourse.masks import make_identity

    f32 = mybir.dt.float32
    bf16 = mybir.dt.bfloat16
    i16 = mybir.dt.int16
    AF = mybir.ActivationFunctionType
    OP = mybir.AluOpType
    AX = mybir.AxisListType

    nc = bacc.Bacc(
        "TRN2",
        target_bir_lowering=False,
        debug=False,
        num_devices=NCORES,
        num_swdge_queues=4,
    )

    x_t = nc.dram_tensor("x", [S, NFEAT], bf16, kind="ExternalInput")
    w1_t = nc.dram_tensor("w1", [NFEAT, NHID], f32, kind="ExternalInput")
    w2_t = nc.dram_tensor("w2", [NHID, NCLASS], f32, kind="ExternalInput")
    b1_t = nc.dram_tensor("b1", [NHID], f32, kind="ExternalInput")
    b2_t = nc.dram_tensor("b2", [NCLASS], f32, kind="ExternalInput")
    dinv_t = nc.dram_tensor("dinv", [P, NT], f32, kind="ExternalInput")
    idx_t = nc.dram_tensor(
        "idxbuf", [16 if IDXB else P, C2], i16, kind="ExternalInput"
    )
    screp_t = nc.dram_tensor("screp", [P, K + 1, NCLASS], f32, kind="ExternalInput")
    sbrep_t = nc.dram_tensor("sbrep", [P, K + 1], f32, kind="ExternalInput")
    t1_t = nc.dram_tensor("t1", [P, K], f32, kind="ExternalInput")
    t2_t = nc.dram_tensor("t2", [P, K], f32, kind="ExternalInput")
    out_t = nc.dram_tensor("out", [S, NCLASS], f32, kind="ExternalOutput")

    def bcast_tail(ap, shape):
        b = ap.to_broadcast(list(shape))
        assert tuple(b.shape) == tuple(shape), (b.shape, shape)
        return b

    def bcast_mid(ap, shape):
        # [128, 1, 64] -> [128, NT, 64]
        try:
            b = ap.to_broadcast(list(shape))
            if tuple(b.shape) == tuple(shape):
                return b
        except Exception:
            pass
        b = ap.broadcast_to(list(shape))
        assert tuple(b.shape) == tuple(shape), (b.shape, shape)
        return b

    with tile.TileContext(nc) as tc:
        with tc.tile_pool(name="persist", bufs=1) as per, tc.tile_pool(
            name="dram", bufs=1, space="DRAM"
        ) as dram:
            inp = per.tile([P, NT, NCLASS], f32)
            hidden = per.tile([P, NT, NCLASS], f32)
            acc = per.tile([P, NT, NCLASS], f32)
            dinv_sb = per.tile([P, NT], f32)
            dinvt_sb = per.tile([P, NT], f32)
            screp_sb = per.tile([P, K + 1, NCLASS], f32)
            sbrep_sb = per.tile([P, K + 1], f32)
            t1_sb = per.tile([P, K], f32)
            t2_sb = per.tile([P, K], f32)
            rows_sb = per.tile([P, NT], f32)
            s_sb = per.tile([P, NT], f32)
            ident = per.tile([P, P], f32)

            nc.sync.dma_start(dinv_sb[:], dinv_t[:])
            nc.sync.dma_start(screp_sb[:], screp_t[:])
            nc.sync.dma_start(sbrep_sb[:], sbrep_t[:])
            nc.sync.dma_start(t1_sb[:], t1_t[:])
            nc.sync.dma_start(t2_sb[:], t2_t[:])
            make_identity(nc, ident[:])

            in_ccq = [
                dram.tile([QS[q], NCLASS], f32, name=f"incc{q}") for q in range(NWIN)
            ]
            tables = [
                [
                    dram.tile(
                        [QR[q], NCLASS], f32, addr_space="Shared",
                        name=f"table{k}_{q}",
                    )
                    for q in range(NWIN)
                ]
                for k in range(K)
            ]

            # ---------------- MLP ----------------
            DO_MLP = DIAG in ("", "mlp_only", "noag")
            DO_GATHER = DIAG in ("", "gather_only2", "fixed_idx", "gather_ag", "noag")
            DO_AG_FIRST = DIAG in ("", "gather_only2", "fixed_idx", "ag_only",
                                   "gather_ag", "noag")
            DO_AG_STEP = DIAG in ("", "ag_only", "gather_ag")
            DO_UPDATE = DIAG in ("", "noag")
            if not DO_MLP:
                nc.vector.memset(inp[:], 0.25)
            if not DO_UPDATE:
                nc.vector.memset(hidden[:], 0.0)
            if not DO_MLP:
                pass
            else:
              with tc.tile_pool(name="mlp", bufs=2) as mp, tc.tile_pool(
                  name="mlpw", bufs=1
              ) as mw, tc.tile_pool(name="psum", bufs=2, space="PSUM") as ps, tc.tile_pool(
                  name="psum2", bufs=2, space="PSUM"
              ) as ps2:
                  # weights -> SBUF bf16
                  w1bf, w2bf = [], []
                  for kc in range(4):
                      wf = mp.tile([P, NHID], f32, tag="wtmp")
                      nc.sync.dma_start(wf[:], w1_t[kc * P : (kc + 1) * P, :])
                      wb = mw.tile([P, NHID], bf16, tag=f"w1b{kc}")
                      nc.vector.tensor_copy(out=wb[:], in_=wf[:])
                      w1bf.append(wb)
                  for mc in range(2):
                      wf = mp.tile([P, NCLASS], f32, tag="wtmp2")
                      nc.sync.dma_start(wf[:], w2_t[mc * P : (mc + 1) * P, :])
                      wb = mw.tile([P, NCLASS], bf16, tag=f"w2b{mc}")
                      nc.vector.tensor_copy(out=wb[:], in_=wf[:])
                      w2bf.append(wb)
                  b1_sb = mw.tile([P, 2], f32, tag="b1")
                  nc.sync.dma_start(b1_sb[:], b1_t[:].rearrange("(m p) -> p m", p=P))
                  b2_sb = mw.tile([NCLASS, 1], f32, tag="b2")
                  nc.sync.dma_start(b2_sb[:], b2_t[:, None])

                  NQ = 1280  # nodes per MLP chunk (10 chunks of 10 tiles)
                  NB = 320   # matmul free block
                  for q in range(S // NQ):
                      xT = []
                      for kc in range(4):
                          xt = mp.tile([P, NQ], bf16, tag=f"xT{kc}", name=f"xT{kc}_{q}")
                          nc.sync.dma_start_transpose(
                              xt[:], x_t[q * NQ : (q + 1) * NQ, kc * P : (kc + 1) * P]
                          )
                          xT.append(xt)
                      h1T = [
                          mp.tile([P, NQ], bf16, tag=f"h1T{mc}", name=f"h1T{mc}_{q}")
                          for mc in range(2)
                      ]
                      for mc in range(2):
                          for nb in range(NQ // NB):
                              pt = ps.tile([P, NB], f32, tag="p1")
                              for kc in range(4):
                                  nc.tensor.matmul(
                                      out=pt[:],
                                      lhsT=w1bf[kc][:, mc * P : (mc + 1) * P],
                                      rhs=xT[kc][:, nb * NB : (nb + 1) * NB],
                                      start=(kc == 0),
                                      stop=(kc == 3),
                                  )
                              nc.scalar.activation(
                                  out=h1T[mc][:, nb * NB : (nb + 1) * NB],
                                  in_=pt[:],
                                  func=AF.Relu,
                                  bias=b1_sb[:, mc : mc + 1],
                              )
                      h2T = mp.tile([NCLASS, NQ], f32, tag="h2T")
                      for nb in range(NQ // NB):
                          pt2 = ps2.tile([NCLASS, NB], f32, tag="p2")
                          for mc in range(2):
                              nc.tensor.matmul(
                                  out=pt2[:],
                                  lhsT=w2bf[mc][:],
                                  rhs=h1T[mc][:, nb * NB : (nb + 1) * NB],
                                  start=(mc == 0),
                                  stop=(mc == 1),
                              )
                          nc.scalar.activation(
                              out=h2T[:, nb * NB : (nb + 1) * NB],
                              in_=pt2[:],
                              func=AF.Identity,
                              bias=b2_sb[:, 0:1],
                          )
                      for tt in range(NQ // P):
                          ptr = ps.tile([P, NCLASS], f32, tag="ptr")
                          nc.tensor.transpose(
                              out=ptr[:],
                              in_=h2T[:, tt * P : (tt + 1) * P],
                              identity=ident[:NCLASS, :NCLASS],
                          )
                          nc.vector.tensor_copy(
                              out=inp[:, q * (NQ // P) + tt, :], in_=ptr[:]
                          )

            # ---------------- propagation ----------------
            rg = [list(range(NCORES))]

            nqt = NQT[0]

            def post_step(k):
                """sigmoid gate with scores[k], update hidden; k=0 init.
                Per-quarter tiles so no whole-tensor scratch is needed."""
                for q in range(NWIN):
                    qsl = slice(q, NT, NWIN)
                    tmp2 = chpool.tile([P, nqt, NCLASS], f32, tag="chtmp2")
                    sc_b = bcast_mid(screp_sb[:, k : k + 1, :], (P, nqt, NCLASS))
                    nc.vector.tensor_tensor(
                        out=tmp2[:], in0=inp[:, qsl, :], in1=sc_b, op=OP.mult
                    )
                    nc.vector.tensor_reduce(
                        out=rows_sb[:, qsl], in_=tmp2[:], axis=AX.X, op=OP.add
                    )
                    nc.scalar.activation(
                        out=s_sb[:, qsl], in_=rows_sb[:, qsl], func=AF.Sigmoid,
                        bias=sbrep_sb[:, k : k + 1],
                    )
                    s_b = bcast_tail(s_sb[:, qsl], (P, nqt, NCLASS))
                    nc.vector.tensor_tensor(
                        out=tmp2[:], in0=inp[:, qsl, :], in1=s_b, op=OP.mult
                    )
                    if k == 0:
                        nc.vector.tensor_copy(out=hidden[:, qsl, :], in_=tmp2[:])
                    else:
                        nc.vector.tensor_add(
                            out=hidden[:, qsl, :], in0=hidden[:, qsl, :], in1=tmp2[:]
                        )

            def emit_table_ag(k):
                # per-quarter scale + AllGather: window q's table only depends
                # on quarter-q rows, so next-step window-q gathers start as
                # soon as AG(q) lands (AGs 1-3 hide behind window-0 gathers)
                for q in range(NWIN):
                    qsl = slice(q, NT, NWIN)
                    tmp = chpool.tile([P, nqt, NCLASS], f32, tag="chtmp")
                    d_b = bcast_tail(dinv_sb[:, qsl], (P, nqt, NCLASS))
                    nc.vector.tensor_tensor(
                        out=tmp[:], in0=inp[:, qsl, :], in1=d_b, op=OP.mult
                    )
                    nc.sync.dma_start(
                        in_ccq[q][:].rearrange("(t p) d -> p t d", p=P), tmp[:]
                    )
                    nc.gpsimd.collective_compute(
                        "AllGather", OP.bypass, replica_groups=rg,
                        ins=[in_ccq[q][:].opt()], outs=[tables[k][q][:].opt()],
                    )

            # pe-mode: slice coverage map for PSUM accumulation groups.
            # cover[(q, s)] = (first_ev, last_ev, full_ns) over (call, level)
            # events touching 8-tile slice s of quarter-chunk q. The
            # (w=0, j=0) level is full width so first_ev always initializes
            # every slice.
            cover = {}
            for ci, (w, q, levels, off) in enumerate(calls):
                for li, (j, klo, khi) in enumerate(levels):
                    for s in range(klo // 8, (khi + 7) // 8):
                        key = (q, s)
                        if key not in cover:
                            cover[key] = [(ci, li), (ci, li), min(8, nqt - 8 * s)]
                        else:
                            cover[key][1] = (ci, li)

            with tc.tile_pool(name="gpool", bufs=GBUFS) as gpool, tc.tile_pool(
                name="ipool", bufs=IBUFS
            ) as ipool, tc.tile_pool(
                name="chpool", bufs=CHB
            ) as chpool, tc.tile_pool(
                name="pspool", bufs=8, space="PSUM"
            ) as pspool:
                if DO_UPDATE:
                    post_step(0)
                if DO_AG_FIRST:
                    emit_table_ag(0)

                for k in range(1, K + 1):
                    if DIAG == "null":
                        break
                    ps_tiles = {}
                    if DO_UPDATE:
                        nc.vector.tensor_scalar(
                            out=dinvt_sb[:], in0=dinv_sb[:],
                            scalar1=t1_sb[:, k - 1 : k], scalar2=None, op0=OP.mult,
                        )
                    for ci, (w, q, levels, off) in enumerate(calls):
                        ntile_call = sum(khi - klo for (j, klo, khi) in levels)
                        if DO_GATHER:
                            ncols = ntile_call * P // 16
                            nidx = ntile_call * P
                            isb = ipool.tile([P, TCH * P // 16], i16, tag="isb")
                            if IDXB:
                                nc.sync.dma_start(
                                    isb[:, :ncols].rearrange(
                                        "(r p) c -> r p c", r=8
                                    ),
                                    idx_t[:, off : off + ncols]
                                    .unsqueeze(0)
                                    .broadcast_to([8, 16, ncols]),
                                )
                            else:
                                nc.sync.dma_start(
                                    isb[:, :ncols], idx_t[:, off : off + ncols]
                                )
                            g = gpool.tile([P, TCH, NCLASS], f32, tag="g")
                            nc.gpsimd.dma_gather(
                                out_ap=g[:, :ntile_call, :],
                                in_ap=tables[(k - 1) if DO_AG_STEP else 0][w][:],
                                idxs_ap=isb[:, :ncols],
                                num_idxs=nidx,
                                num_idxs_reg=nidx,
                                elem_size=NCLASS,
                                single_packet=SINGLE_PACKET,
                                queue_num=ci % 4,
                            )
                            if DO_UPDATE:
                                pos = 0
                                for li, (j, klo, khi) in enumerate(levels):
                                    for s in range(klo // 8, (khi + 7) // 8):
                                        t0 = max(klo, 8 * s)
                                        t1 = min(khi, 8 * s + 8)
                                        ns = t1 - t0
                                        first_ev, last_ev, full_ns = cover[(q, s)]
                                        if first_ev == (ci, li):
                                            pt = pspool.tile(
                                                [P, 8, NCLASS], f32, tag="ps",
                                                name=f"ps_{k}_{q}_{s}",
                                            )
                                            ps_tiles[(q, s)] = pt
                                        pt = ps_tiles[(q, s)]
                                        nc.tensor.matmul(
                                            out=pt[:, t0 - 8 * s : t0 - 8 * s + ns, :],
                                            lhsT=ident[:],
                                            rhs=g[
                                                :, pos + t0 - klo : pos + t0 - klo + ns, :
                                            ],
                                            start=(first_ev == (ci, li)),
                                            stop=(last_ev == (ci, li)),
                                        )
                                        if last_ev == (ci, li):
                                            nc.scalar.activation(
                                                out=acc[
                                                    :,
                                                    q + NWIN * 8 * s : q
                                                    + NWIN * (8 * s + full_ns - 1)
                                                    + 1 : NWIN,
                                                    :,
                                                ],
                                                in_=pt[:, :full_ns, :],
                                                func=AF.Copy,
                                            )
                                    pos += khi - klo
                        # chunk epilogue: last call of quarter q -> update,
                        # AG write + collective, gate/hidden for this quarter
                        # while the next quarter's gathers stream.
                        is_last_of_q = (
                            ci == len(calls) - 1 or calls[ci + 1][1] != q
                        )
                        if DO_UPDATE and is_last_of_q:
                            qsl = slice(q, NT, NWIN)
                            tmp = chpool.tile([P, nqt, NCLASS], f32, tag="chtmp")
                            dt_b = bcast_tail(dinvt_sb[:, qsl], (P, nqt, NCLASS))
                            nc.vector.tensor_tensor(
                                out=tmp[:], in0=acc[:, qsl, :], in1=dt_b, op=OP.mult
                            )
                            nc.vector.tensor_scalar(
                                out=inp[:, qsl, :], in0=inp[:, qsl, :],
                                scalar1=t2_sb[:, k - 1 : k], scalar2=None,
                                op0=OP.mult,
                            )
                            nc.vector.tensor_add(
                                out=inp[:, qsl, :], in0=inp[:, qsl, :], in1=tmp[:]
                            )
                            if DO_AG_STEP and k < K:
                                d_b = bcast_tail(dinv_sb[:, qsl], (P, nqt, NCLASS))
                                nc.vector.tensor_tensor(
                                    out=tmp[:], in0=inp[:, qsl, :], in1=d_b,
                                    op=OP.mult,
                                )
                                nc.sync.dma_start(
                                    in_ccq[q][:].rearrange("(t p) d -> p t d", p=P),
                                    tmp[:],
                                )
                                nc.gpsimd.collective_compute(
                                    "AllGather", OP.bypass, replica_groups=rg,
                                    ins=[in_ccq[q][:].opt()],
                                    outs=[tables[k][q][:].opt()],
                                )
                            # gate + hidden accumulation for this quarter
                            tmp2 = chpool.tile([P, nqt, NCLASS], f32, tag="chtmp2")
                            sc_b = bcast_mid(screp_sb[:, k : k + 1, :], (P, nqt, NCLASS))
                            nc.vector.tensor_tensor(
                                out=tmp2[:], in0=inp[:, qsl, :], in1=sc_b, op=OP.mult
                            )
                            nc.vector.tensor_reduce(
                                out=rows_sb[:, qsl], in_=tmp2[:], axis=AX.X, op=OP.add
                            )
                            nc.scalar.activation(
                                out=s_sb[:, qsl], in_=rows_sb[:, qsl],
                                func=AF.Sigmoid, bias=sbrep_sb[:, k : k + 1],
                            )
                            s_b = bcast_tail(s_sb[:, qsl], (P, nqt, NCLASS))
                            nc.vector.tensor_tensor(
                                out=tmp2[:], in0=inp[:, qsl, :], in1=s_b, op=OP.mult
                            )
                            nc.vector.tensor_add(
                                out=hidden[:, qsl, :], in0=hidden[:, qsl, :],
                                in1=tmp2[:],
                            )

                    if DO_AG_STEP and k < K and not DO_UPDATE:
                        emit_table_ag(k)  # diag-only path (ag_only)

                # ---------------- log_softmax + output ----------------
                for q in range(NWIN):
                    qsl = slice(q, NT, NWIN)
                    tmp = chpool.tile([P, nqt, NCLASS], f32, tag="chtmp")
                    nc.vector.tensor_reduce(
                        out=rows_sb[:, qsl], in_=hidden[:, qsl, :], axis=AX.X,
                        op=OP.max,
                    )
                    m_b = bcast_tail(rows_sb[:, qsl], (P, nqt, NCLASS))
                    nc.vector.tensor_tensor(
                        out=hidden[:, qsl, :], in0=hidden[:, qsl, :], in1=m_b,
                        op=OP.subtract,
                    )
                    nc.scalar.activation(out=tmp[:], in_=hidden[:, qsl, :], func=AF.Exp)
                    nc.vector.tensor_reduce(
                        out=s_sb[:, qsl], in_=tmp[:], axis=AX.X, op=OP.add
                    )
                    nc.scalar.activation(
                        out=s_sb[:, qsl], in_=s_sb[:, qsl], func=AF.Ln
                    )
                    ls_b = bcast_tail(s_sb[:, qsl], (P, nqt, NCLASS))
                    nc.vector.tensor_tensor(
                        out=hidden[:, qsl, :], in0=hidden[:, qsl, :], in1=ls_b,
                        op=OP.subtract,
                    )
                    nc.sync.dma_start(
                        out_t[:].rearrange("(t p) d -> p t d", p=P)[:, qsl, :],
                        hidden[:, qsl, :],
                    )

    nc.compile()
    return nc


# --------------------------------------------------------------------------
# persistent runner (8-core shard_map, reusable device buffers)
# --------------------------------------------------------------------------
def _make_runner(nc, in_maps):
    import jax
    from jax.sharding import Mesh, PartitionSpec
    from jax.experimental.shard_map import shard_map
    from concourse import bass2jax, mybir
    from concourse.bass2jax import _bass_exec_p, install_neuronx_cc_hook

    install_neuronx_cc_hook()
    partition_name = nc.partition_id_tensor.name if nc.partition_id_tensor else None
    in_names, out_names, out_avals = [], [], []
    for alloc in nc.m.functions[0].allocations:
        if not isinstance(alloc, mybir.MemoryLocationSet):
            continue
        name = alloc.memorylocations[0].name
        if alloc.kind == "ExternalInput":
            if name != partition_name:
                in_names.append(name)
        elif alloc.kind == "ExternalOutput":
            out_names.append(name)
            out_avals.append(
                jax.core.ShapedArray(tuple(alloc.tensor_shape), mybir.dt.np(alloc.dtype))
            )
    all_in_names = in_names + out_names + ([partition_name] if partition_name else [])

    def _body(*args):
        operands = list(args)
        if partition_name is not None:
            operands.append(bass2jax.partition_id_tensor())
        return tuple(
            _bass_exec_p.bind(
                *operands,
                out_avals=tuple(out_avals),
                in_names=tuple(all_in_names),
                out_names=tuple(out_names),
                lowering_input_output_aliases=(),
                sim_require_finite=True,
                sim_require_nnan=True,
                nc=nc,
            )
        )

    devices = jax.devices()[:NCORES]
    mesh = Mesh(np.asarray(devices), ("core",))
    nio = len(in_names) + len(out_names)
    fn = jax.jit(
        shard_map(
            _body,
            mesh=mesh,
            in_specs=(PartitionSpec("core"),) * nio,
            out_specs=(PartitionSpec("core"),) * len(out_names),
            check_rep=False,
        ),
        keep_unused=True,
    )
    concat_in = [
        np.concatenate([np.asarray(in_maps[c][n]) for c in range(NCORES)], axis=0)
        for n in in_names
    ]
    concat_zeros = [
        np.zeros((NCORES * a.shape[0], *a.shape[1:]), a.dtype) for a in out_avals
    ]
    args_d = [jax.device_put(x) for x in concat_in + concat_zeros]

    def run(fetch=True):
        out = fn(*args_d)
        jax.block_until_ready(out)
        if not fetch:
            return None
        return {
            n: np.asarray(out[i]).reshape(NCORES, *out_avals[i].shape)
            for i, n in enumerate(out_names)
        }

    return run


# --------------------------------------------------------------------------
# entry point
# --------------------------------------------------------------------------
def kernel(x, edge_index, W1, b1, W2, b2, temp, scores, sbias):
    import hashlib

    ekey = hashlib.md5(np.ascontiguousarray(edge_index)).hexdigest()
    if ekey not in _CACHE:
        pp = _preprocess(np.asarray(edge_index))
        nc = _build_nc(pp["calls"], pp["C2"])
        _CACHE[ekey] = (pp, nc, {})
    pp, nc, runstate = _CACHE[ekey]

    import ml_dtypes

    x = np.asarray(x, np.float32).astype(ml_dtypes.bfloat16)
    TEMP = np.tanh(np.asarray(temp, np.float32))
    scores = np.asarray(scores, np.float32)
    sbias = np.asarray(sbias, np.float32)

    screp = np.tile(scores[None, :, :], (P, 1, 1)).astype(np.float32)
    sbrep = np.tile(sbias[None, :], (P, 1)).astype(np.float32)
    t1 = np.tile(TEMP[None, :], (P, 1)).astype(np.float32)
    t2 = (1.0 - t1).astype(np.float32)

    in_maps = []
    for c in range(NCORES):
        xs = np.zeros((S, NFEAT), ml_dtypes.bfloat16)
        nm = pp["nodemap"][c]
        real = nm >= 0
        xs[real] = x[nm[real]]
        in_maps.append(
            {
                "x": xs,
                "w1": np.asarray(W1, np.float32),
                "w2": np.asarray(W2, np.float32),
                "b1": np.asarray(b1, np.float32),
                "b2": np.asarray(b2, np.float32),
                "dinv": pp["dinv_slot"][c],
                "idxbuf": pp["idxbuf"][c],
                "screp": screp,
                "sbrep": sbrep,
                "t1": t1,
                "t2": t2,
            }
        )

    dkey = hashlib.md5(
        b"".join(np.ascontiguousarray(a) for a in (x[:1000], W1, W2, b1, b2, screp, sbrep, t1))
    ).hexdigest()
    if runstate.get("dkey") != dkey:
        runstate["run"] = _make_runner(nc, in_maps)
        runstate["dkey"] = dkey
        runstate["in_maps"] = in_maps
    res = runstate["run"]()

    out_full = np.empty((N, NCLASS), np.float32)
    for c in range(NCORES):
        nm = pp["nodemap"][c]
        real = nm >= 0
        out_full[nm[real]] = res["out"][c][real]
    return out_full


def benchmark(n_runs=12):
    """Re-execute the resident kernel; returns sorted wall times (s)."""
    import time

    assert _CACHE, "call kernel() first"
    runstate = next(iter(_CACHE.values()))[2]
    run = runstate["run"]
    ts = []
    for _ in range(n_runs):
        t0 = time.perf_counter()
        run(fetch=False)
        ts.append(time.perf_counter() - t0)
    ts.sort()
    return ts

